# revision 1
# baseline (speedup 1.0000x reference)
"""ChebConv (order-4) GNN layer on 8 Trainium2 NeuronCores.

Reference computation (fp32):
    T0 = x, T1 = G x, Tk = 2 G T{k-1} - T{k-2}
    out = sum_k Tk @ W[k]          # [N, F] with N=10000, F=32

Strategy:
  * Rewrite in the power basis: y0 = x, yk = G y{k-1},
      out = sum_k yk @ Wp[k]  with
      Wp = [W0 - W2, W1 - 3 W3, 2 W2, 4 W3]   (exact modulo fp reassociation)
    so each hop is a bare matmul against G (no 2*/- epilogue).
  * Row-shard G over 8 cores (1280 padded rows each). The per-core lhsT
    tiles must hold G^T, so the host passes each core a contiguous
    transposed slice (pad N 10000 -> 10240).
  * fp32 matmuls on the TRN2 PE run in LOW_HIGH mode: 2 passes, each
    streaming the fp32 rhs at half rate (4x bf16 cost). Instead we do a
    software hi/lo split: G = G_hi + G_lo and v = v_hi + v_lo (bf16
    pairs) and compute G_hi v_hi + G_lo v_hi + G_hi v_lo with fp32 PSUM
    accumulation -- 3 full-rate bf16 passes, same DRAM bytes as fp32,
    ~7e-6 relative error (vs 3e-3 for plain bf16). Per fc sweep, G_hi
    and G_lo rows are interleaved in one [NP, 2*l] array so each
    128-row j-chunk is a single contiguous DMA.
  * Each hop runs as 3 sweeps, one per <=512-column chunk of yk^T.
    Per sweep and 128-row j-chunk: matmuls (lhsT=v_{hi,lo}[j-chunk]
    [128,32] bf16, rhs=G^T_{hi,lo} tile [128,<=512] bf16) accumulate
    the sweep's [32,<=512] chunk of yk^T over all 80 j-chunks (one open
    PSUM accumulation group per bank). The last (smallest) sweep's G
    block stays pinned in SBUF across hops (10.5 MB saved twice).
  * The Wp contraction happens on-chip from yk^T in full fp32:
    matmul(lhsT=Wp_k [32,32], rhs=ykT chunk), DVE-add into the
    transposed output accumulator; the k=0 term uses the host xT slice.
  * After each sweep (except in the last hop), its rows are
    PE-transposed ([32,128] -> [128,32] blocks) into natural m-chunk
    layout, split into bf16 hi/lo, and all-gathered in a partial
    collective (DRAM bounce) that overlaps the remaining sweeps. The
    reload into the next hop's per-part v tiles rides SWDGE (gpsimd) so
    the gather-gated DMA cannot convoy the G stream on the shared HWDGE
    completion lanes; j-chunks are consumed in gather-firing order so
    each hop starts on columns whose gather finished first.
  * Output is returned transposed ([32, 1280] per core); the host
    concatenates, transposes and drops padding.
"""

import sys

if "/opt/trn_rl_repo" not in sys.path:
    sys.path.insert(0, "/opt/trn_rl_repo")

import numpy as np

N = 10000
F = 32
ORDER = 4
NCORES = 8
P = 128
NP = 10240  # padded node count: divisible by NCORES * P
RPC = NP // NCORES  # rows per core (1280)
JC = NP // P  # global 128-row chunks (80)
MC = RPC // P  # local 128-row chunks per core (10)

_CACHE = {}


def _build(np_total, ncores):
    from concourse import bacc, masks, mybir, tile

    rpc = np_total // ncores
    jc = np_total // P
    mc = rpc // P
    f32 = mybir.dt.float32
    bf16 = mybir.dt.bfloat16
    fchunks = [(s, min(512, rpc - s)) for s in range(0, rpc, 512)]
    nfc = len(fchunks)

    nc = bacc.Bacc(
        "TRN2", target_bir_lowering=False, debug=False, num_devices=ncores
    )
    # one G^T block per fc sweep, rows = [hi cols | lo cols] interleaved
    ghls = [
        nc.dram_tensor(f"ghl{i}", [np_total, 2 * l], bf16, kind="ExternalInput").ap()
        for i, (s, l) in enumerate(fchunks)
    ]
    # per-part m-chunk geometry: part i covers m-chunks [m0, m0+nm)
    parts = [(s // P, l // P) for s, l in fchunks]
    # x in per-part v layout: concat over parts of [hi block | lo block],
    # block col (c*nm + ml)*F + f = padded x row (c*mc + m0 + ml)*P + p
    vcols = [2 * ncores * nm * F for (m0, nm) in parts]
    xthl = nc.dram_tensor("xthl", [P, sum(vcols)], bf16, kind="ExternalInput").ap()
    xt = nc.dram_tensor("xt", [F, rpc], f32, kind="ExternalInput").ap()
    wp = nc.dram_tensor("wp", [F, ORDER * F], f32, kind="ExternalInput").ap()
    out_t = nc.dram_tensor("outT", [F, rpc], f32, kind="ExternalOutput").ap()

    # pin the last (smallest) fc sweep's G block in SBUF across hops
    pin_i = nfc - 1
    pin_l = fchunks[pin_i][1]

    def part_of(m):
        for i, (m0, nm) in enumerate(parts):
            if m0 <= m < m0 + nm:
                return i
        raise AssertionError

    with tile.TileContext(nc) as tc:
        with (
            tc.tile_pool(name="const", bufs=1) as constp,
            tc.tile_pool(name="gtp", bufs=10) as gtp,
            tc.tile_pool(name="vp", bufs=2) as vp,
            tc.tile_pool(name="sb", bufs=2) as sb,
            tc.tile_pool(name="ps_hop", bufs=1, space="PSUM") as ps_hop,
            tc.tile_pool(name="ps_tp", bufs=2, space="PSUM") as ps_tp,
            tc.tile_pool(name="ps_w", bufs=2, space="PSUM") as ps_w,
            tc.tile_pool(name="dram", bufs=2, space="DRAM") as dram,
        ):
            ident = constp.tile([P, P], f32)
            masks.make_identity(nc, ident[:])
            w_sb = constp.tile([F, ORDER * F], f32)
            nc.scalar.dma_start(w_sb[:], wp)
            xt_sb = constp.tile([F, rpc], f32)
            nc.scalar.dma_start(xt_sb[:], xt)
            out_sb = constp.tile([F, rpc], f32)
            pin = constp.tile([P, jc * 2 * pin_l], bf16)

            # v holds y_{k-1} as bf16 hi/lo pairs, one tile per fc part so
            # next-hop matmuls only depend on the partial gather that
            # produced their columns
            v_parts = []
            off = 0
            for i, w_ in enumerate(vcols):
                vt = vp.tile([P, w_], bf16, tag=f"v{i}", name=f"v{i}")
                nc.scalar.dma_start(vt[:], xthl[:, off : off + w_])
                off += w_
                v_parts.append(vt)

            def v_hi(vps, j):
                c, m = j // mc, j % mc
                i = part_of(m)
                m0, nm = parts[i]
                col = (c * 2 * nm + (m - m0)) * F
                return vps[i][:, col : col + F]

            def v_lo(vps, j):
                c, m = j // mc, j % mc
                i = part_of(m)
                m0, nm = parts[i]
                col = (c * 2 * nm + nm + (m - m0)) * F
                return vps[i][:, col : col + F]

            # k = 0 contribution: out^T = Wp_0^T @ x^T (pure fp32)
            for s, l in fchunks:
                pw = ps_w.tile([F, l], f32, tag="pw")
                nc.tensor.matmul(
                    pw[:], lhsT=w_sb[:, 0:F], rhs=xt_sb[:, s : s + l],
                    start=True, stop=True,
                )
                nc.vector.tensor_copy(out_sb[:, s : s + l], pw[:])

            # j-chunks are consumed in sweep (= gather-firing) order so
            # each hop starts on columns whose gather finished first; the
            # pinned sweep stays last: its gather is smallest and its
            # consumers come after a ~46us runway in the next hop
            sweep_order = list(range(nfc))
            part_rank = {i: r for r, i in enumerate(sweep_order)}
            jorder = sorted(range(jc), key=lambda j: (part_rank[part_of(j % mc)], j))

            def reload_v(i, cc_out, v_dst):
                # SWDGE (gpsimd) so the gather-gated reload can't convoy
                # the G stream on the shared HWDGE completion lanes; one
                # DMA per part (hi/lo interleaved per core block)
                nc.gpsimd.dma_start(
                    v_dst[i][:].rearrange("p (c m) -> p c m", c=ncores),
                    cc_out[:].rearrange("(c p) m -> p c m", p=P),
                )

            for k in range(1, ORDER):
                v_cur = v_parts
                if k < ORDER - 1:
                    v_next = [
                        vp.tile([P, w_], bf16, tag=f"v{i}", name=f"vn{i}")
                        for i, w_ in enumerate(vcols)
                    ]
                y_t = sb.tile([F, rpc], f32, tag="yT")
                js = jorder
                # hop: y_k^T = (G @ y_{k-1})^T via 3 bf16 hi/lo passes,
                # one sweep per fc chunk so partial all-gathers overlap
                # the remaining sweeps
                for i in sweep_order:
                    s, l = fchunks[i]
                    # when both hi and lo rhs fit one PSUM bank, fuse the
                    # two v_hi passes into a single 2l-column matmul and
                    # fold the halves with the epilogue DVE op instead
                    merged = 2 * l <= 512
                    hp = ps_hop.tile(
                        [F, 2 * l] if merged else [F, l],
                        f32, tag=f"hop{i}", name=f"hp{i}",
                    )
                    pinned = i == pin_i
                    for jn, j in enumerate(js):
                        if pinned:
                            g = pin[:, j * 2 * l : (j + 1) * 2 * l]
                            if k == 1:
                                nc.sync.dma_start(
                                    g, ghls[i][j * P : (j + 1) * P, :]
                                )
                        else:
                            gt = gtp.tile(
                                [P, 2 * l], bf16, tag=f"gt{i}", name="gt"
                            )
                            nc.sync.dma_start(
                                gt[:], ghls[i][j * P : (j + 1) * P, :]
                            )
                            g = gt[:]
                        gh = g[:, 0:l]
                        gl = g[:, l : 2 * l]
                        if merged:
                            nc.tensor.matmul(
                                hp[:], lhsT=v_hi(v_cur, j), rhs=g[:, 0 : 2 * l],
                                start=(jn == 0), stop=False,
                            )
                            nc.tensor.matmul(
                                hp[:, 0:l], lhsT=v_lo(v_cur, j), rhs=gh,
                                start=False, stop=(jn == jc - 1),
                            )
                        else:
                            for t, (lhs, rhs) in enumerate(
                                (
                                    (v_hi(v_cur, j), gh),
                                    (v_lo(v_cur, j), gh),
                                    (v_hi(v_cur, j), gl),
                                )
                            ):
                                nc.tensor.matmul(
                                    hp[:], lhsT=lhs, rhs=rhs,
                                    start=(jn == 0 and t == 0),
                                    stop=(jn == jc - 1 and t == 2),
                                )
                    # sweep epilogue: copy out (folding the merged
                    # halves), Wp contribution
                    if merged:
                        # walrus allows only one PSUM operand per DVE op
                        nc.vector.tensor_copy(y_t[:, s : s + l], hp[:, 0:l])
                        nc.vector.tensor_add(
                            y_t[:, s : s + l], y_t[:, s : s + l], hp[:, l : 2 * l]
                        )
                    else:
                        nc.vector.tensor_copy(y_t[:, s : s + l], hp[:])
                    pw = ps_w.tile([F, l], f32, tag="pw")
                    nc.tensor.matmul(
                        pw[:], lhsT=w_sb[:, k * F : (k + 1) * F],
                        rhs=y_t[:, s : s + l], start=True, stop=True,
                    )
                    nc.vector.tensor_add(
                        out_sb[:, s : s + l], out_sb[:, s : s + l], pw[:]
                    )
                    if k < ORDER - 1:
                        # transpose this sweep's rows to natural layout,
                        # split bf16 hi/lo, partial all-gather; the
                        # reload into the next hop's v happens there
                        m0, nm = parts[i]
                        stage = sb.tile(
                            [P, 2 * nm * F], bf16, tag=f"stage{i}",
                            name=f"stage{i}",
                        )
                        for mm in range(nm):
                            m = m0 + mm
                            tp = ps_tp.tile([P, F], f32, tag="tp", name="tp")
                            nc.tensor.transpose(
                                tp[:], y_t[:, m * P : (m + 1) * P],
                                ident[0:F, 0:F],
                            )
                            hi = stage[:, mm * F : (mm + 1) * F]
                            lo = stage[:, (nm + mm) * F : (nm + mm + 1) * F]
                            nc.vector.tensor_copy(hi, tp[:])
                            nc.vector.tensor_sub(lo, tp[:], hi)
                        cc_in = dram.tile(
                            [P, 2 * nm * F], bf16, tag=f"ccin{i}",
                            name=f"ccin{i}",
                        )
                        cc_out = dram.tile(
                            [ncores * P, 2 * nm * F], bf16, tag=f"ccout{i}",
                            name=f"ccout{i}",
                        )
                        nc.scalar.dma_start(cc_in[:], stage[:])
                        nc.gpsimd.collective_compute(
                            "AllGather",
                            mybir.AluOpType.bypass,
                            replica_groups=[list(range(ncores))],
                            ins=[cc_in.opt()],
                            outs=[cc_out.opt()],
                        )
                        reload_v(i, cc_out, v_next)
                if k < ORDER - 1:
                    v_parts = v_next

            nc.scalar.dma_start(out_t, out_sb[:])

    nc.compile()
    return nc


def get_nc(np_total=NP, ncores=NCORES):
    key = (np_total, ncores)
    if key not in _CACHE:
        _CACHE[key] = _build(np_total, ncores)
    return _CACHE[key]


def _bf16_pair(a):
    import ml_dtypes

    hi = a.astype(ml_dtypes.bfloat16)
    lo = (a - hi.astype(np.float32)).astype(ml_dtypes.bfloat16)
    return hi, lo


def prep_inputs(x, gso, weight, np_total=NP, ncores=NCORES):
    """Host-side shard prep. Returns in_maps for run_bass_kernel_spmd."""
    n = x.shape[0]
    rpc = np_total // ncores
    jc = np_total // P

    x = np.asarray(x, dtype=np.float32)
    gso = np.asarray(gso, dtype=np.float32)
    weight = np.asarray(weight, dtype=np.float32)

    wp = np.concatenate(
        [
            weight[0] - weight[2],
            weight[1] - 3.0 * weight[3],
            2.0 * weight[2],
            4.0 * weight[3],
        ],
        axis=1,
    ).astype(np.float32)  # [F, ORDER*F]

    xpad = np.zeros((np_total, F), dtype=np.float32)
    xpad[:n] = x
    gpad = np.zeros((np_total, np_total), dtype=np.float32)
    gpad[:n, :n] = gso
    g_hi, g_lo = _bf16_pair(gpad)

    # x as bf16 hi/lo pair in the per-part v layout:
    # for part (m0, nm): block col (c*nm + ml)*F + f = row (c*mc+m0+ml)*P + p
    x_hi, x_lo = _bf16_pair(xpad)
    mc = rpc // P
    parts = [(s // P, min(512, rpc - s) // P) for s in range(0, rpc, 512)]

    def part_x(m0, nm):
        # [P, (c, hi|lo, ml, f)] interleaved per core block
        hi = x_hi.reshape(ncores, mc, P, F)[:, m0 : m0 + nm].transpose(2, 0, 1, 3)
        lo = x_lo.reshape(ncores, mc, P, F)[:, m0 : m0 + nm].transpose(2, 0, 1, 3)
        return np.stack([hi, lo], axis=2).reshape(P, ncores * 2 * nm * F)

    xthl = np.ascontiguousarray(
        np.concatenate([part_x(m0, nm) for (m0, nm) in parts], axis=1)
    )

    fchunks = [(s, min(512, rpc - s)) for s in range(0, rpc, 512)]
    in_maps = []
    for c in range(ncores):
        rows = slice(c * rpc, (c + 1) * rpc)
        ght_c = g_hi[rows, :].T  # [np_total, rpc] bf16
        glt_c = g_lo[rows, :].T
        m = {"xthl": xthl, "wp": wp}
        m["xt"] = np.ascontiguousarray(xpad[rows, :].T)  # [F, rpc] fp32
        for i, (s, l) in enumerate(fchunks):
            # per-row [hi cols | lo cols] for this fc sweep
            m[f"ghl{i}"] = np.ascontiguousarray(
                np.concatenate(
                    [ght_c[:, s : s + l], glt_c[:, s : s + l]], axis=1
                )
            )
        in_maps.append(m)
    return in_maps


def assemble_output(results, n=N, ncores=NCORES):
    out_t = np.concatenate([results[c]["outT"] for c in range(ncores)], axis=1)
    return np.ascontiguousarray(out_t.T[:n]).astype(np.float32)


def kernel(x, gso, weight):
    import time

    from concourse import bass_utils

    nc = get_nc()
    in_maps = prep_inputs(x, gso, weight)
    last_err = None
    for attempt in range(3):
        try:
            res = bass_utils.run_bass_kernel_spmd(
                nc, in_maps, core_ids=list(range(NCORES))
            )
            return assemble_output(res.results)
        except Exception as e:  # transient device wedge: retry
            last_err = e
            time.sleep(5.0 * (attempt + 1))
    raise last_err



# revision 3
# speedup vs baseline: 2.1189x; 2.1189x over previous
"""ChebConv (order-4) GNN layer on 8 Trainium2 NeuronCores.

Reference computation (fp32):
    T0 = x, T1 = G x, Tk = 2 G T{k-1} - T{k-2}
    out = sum_k Tk @ W[k]          # [N, F] with N=10000, F=32

Strategy (v2 — plain bf16 + 4x col-tiled PE + pinned G):
  * Power basis: y0 = x, yk = G y{k-1}, out = sum_k yk @ Wp[k] with
    Wp = [W0 - W2, W1 - 3 W3, 2 W2, 4 W3] (exact modulo fp reassociation).
  * Everything in plain bf16 (G, x, y between hops, Wp); fp32 PSUM
    accumulation. Measured end-to-end rel err ~4e-3 vs the 2e-2 gate;
    halves DRAM bytes and cuts matmul passes 3x vs the hi/lo split.
  * Row-shard G over 8 cores (1280 cols of G^T per core, padded
    N 10000 -> 10240). Contraction j-chunks: 79 of 80 (last is all-pad).
  * Per hop, y^T accumulates per <=512-col sweep: sweeps (512,512,256).
    Sweeps 0,1 (1024 cols) of G^T stay PINNED in SBUF across all hops
    (~158 KB/partition); sweep 2 streams from DRAM every hop,
    interleaved 2:1 between pinned groups so the DMA stream is consumed
    uniformly across the hop instead of in an end-of-hop burst.
  * Matmuls are 4x column-tiled: groups of 4 j-chunks run concurrently
    in the 128x128 PE array (lhsT = v_j [128,32] at col-group 32t, rhs
    = G^T tile [128,l]), accumulating into 4 disjoint 32-partition
    strips of one PSUM bank. DVE reduces the 4 strips (fp32) and casts
    to bf16 y_t. ~4x PE throughput at M=32.
  * G^T rows are host-permuted into consumption order (parts 0,2,1 to
    match gather completion order), so each 4-j group is one contiguous
    DMA descriptor.
  * The Wp contraction accumulates in PSUM across all hops (k=0 term
    from x^T; one matmul per sweep per hop; stop at k=3), copied to
    SBUF once at the end.
  * After each sweep (hops 1,2), its y rows are PE-transposed to
    natural layout, staged bf16, all-gathered (DRAM bounce), and
    reloaded into the next hop's per-part v tiles via SWDGE (gpsimd)
    so the gather-gated DMA cannot convoy the G stream.
  * Output is returned transposed ([32, 1280] per core); the host
    concatenates, transposes and drops padding.
"""

import sys

if "/opt/trn_rl_repo" not in sys.path:
    sys.path.insert(0, "/opt/trn_rl_repo")

import numpy as np

N = 10000
F = 32
ORDER = 4
NCORES = 8
P = 128
NP = 10240  # padded node count
RPC = NP // NCORES  # cols of G^T per core (1280)
MC = RPC // P  # m-chunks per core (10)
JC_EFF = 79  # 128-row contraction chunks with any real data (80th is pad)
SWEEPS = [(0, 512), (512, 512), (1024, 256)]
PARTS = [(0, 4), (4, 4), (8, 2)]  # (m0, nm) per sweep
CONS_ORDER = [0, 2, 1]  # part consumption order (matches gather arrival)

_CACHE = {}


def _part_of(m):
    for i, (m0, nm) in enumerate(PARTS):
        if m0 <= m < m0 + nm:
            return i
    raise AssertionError


def _jlist_groups():
    """Consumption-ordered j list and 4-j groups (never spanning parts)."""
    jlist = []
    groups = []  # (jstart_idx, size, part)
    for i in CONS_ORDER:
        m0, nm = PARTS[i]
        pj = [
            c * MC + m
            for c in range(NCORES)
            for m in range(m0, m0 + nm)
            if c * MC + m < JC_EFF
        ]
        for a in range(0, len(pj), 4):
            chunk = pj[a : a + 4]
            groups.append((len(jlist) + a, len(chunk), i))
        jlist.extend(pj)
    assert len(jlist) == JC_EFF
    return jlist, groups


JLIST, GROUPS = _jlist_groups()
NG = len(GROUPS)  # 20


def _build():
    import heapq

    from concourse import bacc, mybir, tile

    f32 = mybir.dt.float32
    bf16 = mybir.dt.bfloat16
    vcols = [NCORES * nm * F for (_, nm) in PARTS]

    nc = bacc.Bacc(
        "TRN2", target_bir_lowering=False, debug=False, num_devices=NCORES
    )
    g_dram = [
        nc.dram_tensor(f"g{i}", [JC_EFF * P, l], bf16, kind="ExternalInput").ap()
        for i, (_, l) in enumerate(SWEEPS)
    ]
    xtv = nc.dram_tensor("xtv", [P, sum(vcols)], bf16, kind="ExternalInput").ap()
    xt = nc.dram_tensor("xt", [F, RPC], bf16, kind="ExternalInput").ap()
    wp = nc.dram_tensor("wp", [F, ORDER * F], bf16, kind="ExternalInput").ap()
    ident = nc.dram_tensor("ident", [F, F], bf16, kind="ExternalInput").ap()
    out_t = nc.dram_tensor("outT", [F, RPC], f32, kind="ExternalOutput").ap()

    with tile.TileContext(nc) as tc:
        with (
            tc.tile_pool(name="const", bufs=1) as constp,
            tc.tile_pool(name="g2p", bufs=6) as g2p,
            tc.tile_pool(name="vp", bufs=2) as vp,
            tc.tile_pool(name="sb", bufs=2) as sb,
            tc.tile_pool(name="tmp", bufs=1) as tmpp,
            tc.tile_pool(name="ps_hop", bufs=1, space="PSUM") as ps_hop,
            tc.tile_pool(name="ps_w", bufs=1, space="PSUM") as ps_w,
            tc.tile_pool(name="ps_tp", bufs=2, space="PSUM") as ps_tp,
            tc.tile_pool(name="dram", bufs=2, space="DRAM") as dram,
        ):
            w_sb = constp.tile([F, ORDER * F], bf16)
            nc.scalar.dma_start(w_sb[:], wp)
            id_sb = constp.tile([F, F], bf16)
            nc.scalar.dma_start(id_sb[:], ident)
            xt_sb = constp.tile([F, RPC], bf16)
            nc.scalar.dma_start(xt_sb[:], xt)
            out_sb = constp.tile([F, RPC], f32)
            pins = [
                constp.tile([P, JC_EFF * l], bf16, name=f"pin{i}")
                for i, (_, l) in enumerate(SWEEPS[:2])
            ]

            # initial v (= x) in per-part layout
            v_cur = []
            off = 0
            for i, w_ in enumerate(vcols):
                vt = vp.tile([P, w_], bf16, tag=f"v{i}", name=f"v{i}")
                nc.scalar.dma_start(vt[:], xtv[:, off : off + w_])
                off += w_
                v_cur.append(vt)

            def v_of(vtiles, j):
                c, m = j // MC, j % MC
                i = _part_of(m)
                m0, nm = PARTS[i]
                col = (c * nm + (m - m0)) * F
                return vtiles[i][:, col : col + F]

            # Wp output accumulators: one PSUM bank per sweep range, one
            # accumulation group spanning the whole kernel (k = 0..3)
            pw = [
                ps_w.tile([F, l], f32, tag=f"pw{i}", name=f"pw{i}")
                for i, (_, l) in enumerate(SWEEPS)
            ]
            for i, (s, l) in enumerate(SWEEPS):
                nc.tensor.matmul(
                    pw[i][:], lhsT=w_sb[:, 0:F], rhs=xt_sb[:, s : s + l],
                    start=True, stop=False,
                )

            # ---- slot machine: pending epilogues fire by slot index ----
            slot = 0
            seq = 0
            pending = []  # heap of (due_slot, seq, fn)

            def queue(due, fn):
                nonlocal seq
                heapq.heappush(pending, (due, seq, fn))
                seq += 1

            def flush(limit=None):
                while pending and (limit is None or pending[0][0] <= limit):
                    _, _, fn = heapq.heappop(pending)
                    fn()

            def emit_group(hp_, vtiles, i, g, gt=None):
                s, l = SWEEPS[i]
                jstart, sz, _ = GROUPS[g]
                for t in range(sz):
                    q = jstart + t
                    if gt is None:
                        rhs = pins[i][:, q * l : (q + 1) * l]
                    else:
                        rhs = gt[:, t * l : (t + 1) * l]
                    nc.tensor.matmul(
                        hp_[i][32 * t : 32 * (t + 1), 0:l],
                        lhsT=v_of(vtiles, JLIST[q]),
                        rhs=rhs,
                        start=(g == 0),
                        stop=(g == NG - 1),
                        tile_position=(0, 32 * t),
                        skip_group_check=True,
                    )

            def epi_a(i, hp_, y_t_):
                s, l = SWEEPS[i]
                yt_f = tmpp.tile([F, l], f32, tag=f"ytmp{i}", name=f"ytmp{i}")
                nc.vector.tensor_copy(yt_f[:], hp_[i][0:32, 0:l])
                for t in range(1, 4):
                    nc.vector.tensor_add(
                        yt_f[:], yt_f[:], hp_[i][32 * t : 32 * (t + 1), 0:l]
                    )
                nc.vector.tensor_copy(y_t_[:, s : s + l], yt_f[:])

            def epi_b(i, kk, y_t_, stages_):
                s, l = SWEEPS[i]
                if stages_ is not None:
                    m0, nm = PARTS[i]
                    for ml in range(nm):
                        m = m0 + ml
                        tp = ps_tp.tile([P, F], bf16, tag="tp", name="tp")
                        nc.tensor.transpose(
                            tp[:], y_t_[:, m * P : (m + 1) * P], id_sb[:]
                        )
                        nc.vector.tensor_copy(
                            stages_[i][:, ml * F : (ml + 1) * F], tp[:]
                        )
                nc.tensor.matmul(
                    pw[i][:],
                    lhsT=w_sb[:, kk * F : (kk + 1) * F],
                    rhs=y_t_[:, s : s + l],
                    start=False,
                    stop=(kk == ORDER - 1),
                )

            def epi_c(i, v_next_, stages_):
                nm = PARTS[i][1]
                cc_in = dram.tile(
                    [P, nm * F], bf16, tag=f"ccin{i}", name=f"ccin{i}"
                )
                nc.scalar.dma_start(cc_in[:], stages_[i][:])
                cc_out = dram.tile(
                    [NCORES * P, nm * F], bf16, tag=f"ccout{i}", name=f"ccout{i}"
                )
                nc.gpsimd.collective_compute(
                    "AllGather",
                    mybir.AluOpType.bypass,
                    replica_groups=[list(range(NCORES))],
                    ins=[cc_in.opt()],
                    outs=[cc_out.opt()],
                )
                nc.gpsimd.dma_start(
                    v_next_[i][:].rearrange("p (c m) -> p c m", c=NCORES),
                    cc_out[:].rearrange("(c p) m -> p c m", p=P),
                )

            def sweep_done(i, kk, hp_, y_t_, v_next_, stages_):
                # reduce now; transposes/Wp a bit later; gather after
                epi_a(i, hp_, y_t_)
                st = stages_ if kk < ORDER - 1 else None
                queue(
                    slot + 5,
                    lambda i=i, kk=kk, y=y_t_, st=st: epi_b(i, kk, y, st),
                )
                if kk < ORDER - 1:
                    queue(
                        slot + 6,
                        lambda i=i, vn=v_next_, st=st: epi_c(i, vn, st),
                    )

            for k in range(1, ORDER):
                y_t = sb.tile([F, RPC], bf16, tag="yt", name=f"yt{k}")
                hp = [
                    ps_hop.tile([P, l], f32, tag=f"hp{i}", name=f"hp{i}k{k}")
                    for i, (_, l) in enumerate(SWEEPS)
                ]
                last = k == ORDER - 1
                vn = stg = None
                if not last:
                    vn = [
                        vp.tile([P, w_], bf16, tag=f"v{i}", name=f"vn{i}k{k}")
                        for i, w_ in enumerate(vcols)
                    ]
                    stg = [
                        sb.tile(
                            [P, nm * F], bf16, tag=f"stage{i}", name=f"st{i}k{k}"
                        )
                        for i, (_, nm) in enumerate(PARTS)
                    ]

                pg = 0  # pinned group cursor: sweep 0 groups then sweep 1
                for tri in range(NG):
                    for _ in range(2):
                        i, g = (0, pg) if pg < NG else (1, pg - NG)
                        s, l = SWEEPS[i]
                        jstart, sz, _ = GROUPS[g]
                        if k == 1:
                            dst = pins[i][
                                :, jstart * l : (jstart + sz) * l
                            ].rearrange("p (s l) -> p s l", s=sz)
                            src = g_dram[i][
                                jstart * P : (jstart + sz) * P, :
                            ].rearrange("(s p) l -> p s l", p=P)
                            nc.sync.dma_start(dst, src)
                        emit_group(hp, v_cur, i, g)
                        slot += 1
                        if g == NG - 1:
                            sweep_done(i, k, hp, y_t, vn, stg)
                        pg += 1
                        flush(slot)
                    # streamed sweep-2 group
                    s2, l2 = SWEEPS[2]
                    jstart, sz, _ = GROUPS[tri]
                    gt = g2p.tile([P, 4 * l2], bf16, tag="g2", name=f"g2k{k}")
                    nc.sync.dma_start(
                        gt[:, 0 : sz * l2].rearrange("p (s l) -> p s l", s=sz),
                        g_dram[2][
                            jstart * P : (jstart + sz) * P, :
                        ].rearrange("(s p) l -> p s l", p=P),
                    )
                    emit_group(hp, v_cur, 2, tri, gt=gt)
                    slot += 1
                    if tri == NG - 1:
                        sweep_done(2, k, hp, y_t, vn, stg)
                    flush(slot)
                if not last:
                    v_cur = vn

            flush()  # remaining epilogues (hop-3 tail)
            for i, (s, l) in enumerate(SWEEPS):
                nc.vector.tensor_copy(out_sb[:, s : s + l], pw[i][:])
            nc.scalar.dma_start(out_t, out_sb[:])

    nc.compile()
    return nc


def get_nc():
    if "nc" not in _CACHE:
        _CACHE["nc"] = _build()
    return _CACHE["nc"]


def prep_inputs(x, gso, weight):
    """Host-side shard prep. Returns in_maps for run_bass_kernel_spmd."""
    import ml_dtypes

    bf = ml_dtypes.bfloat16
    n = x.shape[0]
    x = np.asarray(x, dtype=np.float32)
    gso = np.asarray(gso, dtype=np.float32)
    weight = np.asarray(weight, dtype=np.float32)

    wp = np.concatenate(
        [
            weight[0] - weight[2],
            weight[1] - 3.0 * weight[3],
            2.0 * weight[2],
            4.0 * weight[3],
        ],
        axis=1,
    ).astype(bf)  # [F, ORDER*F]

    xpad = np.zeros((NP, F), dtype=np.float32)
    xpad[:n] = x
    x_bf = xpad.astype(bf)
    gpad = np.zeros((NP, NP), dtype=np.float32)
    gpad[:n, :n] = gso
    g_bf = gpad.astype(bf)

    jrows = np.concatenate(
        [np.arange(j * P, (j + 1) * P) for j in JLIST]
    )  # [JC_EFF*P]

    # x in per-part v layout: part i -> [P, (c, ml, f)]
    xr = x_bf.reshape(NCORES, MC, P, F)
    xtv = np.concatenate(
        [
            np.ascontiguousarray(
                xr[:, m0 : m0 + nm].transpose(2, 0, 1, 3)
            ).reshape(P, NCORES * nm * F)
            for (m0, nm) in PARTS
        ],
        axis=1,
    )

    ident = np.eye(F, dtype=bf)
    in_maps = []
    for c in range(NCORES):
        rows = slice(c * RPC, (c + 1) * RPC)
        # G^T block with contraction rows permuted into consumption order
        ght_p = np.ascontiguousarray(g_bf[rows][:, jrows].T)  # [JC_EFF*P, RPC]
        m = {"xtv": xtv, "wp": wp, "ident": ident}
        m["xt"] = np.ascontiguousarray(x_bf[rows].T)  # [F, RPC]
        for i, (s, l) in enumerate(SWEEPS):
            m[f"g{i}"] = np.ascontiguousarray(ght_p[:, s : s + l])
        in_maps.append(m)
    return in_maps


def assemble_output(results, n=N, ncores=NCORES):
    out_t = np.concatenate([results[c]["outT"] for c in range(ncores)], axis=1)
    return np.ascontiguousarray(out_t.T[:n]).astype(np.float32)


def kernel(x, gso, weight):
    import time

    from concourse import bass_utils

    nc = get_nc()
    in_maps = prep_inputs(x, gso, weight)
    last_err = None
    for attempt in range(3):
        try:
            res = bass_utils.run_bass_kernel_spmd(
                nc, in_maps, core_ids=list(range(NCORES))
            )
            return assemble_output(res.results)
        except Exception as e:  # transient device wedge: retry
            last_err = e
            time.sleep(5.0 * (attempt + 1))
    raise last_err
